# revision 19
# baseline (speedup 1.0000x reference)
"""CAMSA multi-mask attention kernel for one TRN2 chip (8 NeuronCores).

Problem: B=4, S=2048, D=1024, M=4 stride masks.
  Q = x@Wq + bq ; K = x@Wk + bk ; V = x@Wv + bv     (biases are zero-fill)
  scores = Q K^T / sqrt(D)                           [B,S,S]
  weights_m = softmax(where(mask_m==0, -1e9, scores))
  out = (mean_m weights_m) @ V @ Wo + bo

Algebra: with P = exp(scores/sqrt(D)) (no row-max needed; scores ~ N(0,1)):
  den_m[q] = sum_k mask_m[q,k] P[q,k];  inv_m = 1/den_m
  Wsum = sum_m inv_m * (mask_m*P);  out = Wsum @ V @ (Wo/M)
(the 1/M mean is folded into Wo on the host).

Sharding: core c = (batch b=c//2, query-half h=c%2): 1024 query rows,
full 2048 keys; K/V projections duplicated within a batch pair (no
collectives).  Host-side prep is pure dtype/layout: bf16 casts of
x/W (the device would DMA-cast anyway), masks int32 -> uint8 in a
per-q-tile layout, and a per-core "own half first" permutation of the
key axis (applied consistently to x columns and mask k) so one SPMD
program serves both halves without a separate xTq input.

Device pipeline per core (all matmuls bf16, contraction on partitions):
  tensor order: Q proj, K proj, V proj, scores t=0..7, AV 2-tile
  groups interleaved with the Wo projection per q-tile.
  per q-tile softmax chain under the matmul stream:
    ACT: P=exp(scores) from PSUM (4 blocks), 2 scaled copies
    DVE: 3x STT(mask*P, accum->den), recip, 2x TS(4x), 3x TT(2x)
    GpSimd: 1x STT(mask*P, accum->den)
    sync-DMA: Wsum -> WT transpose (xbar), outputs
"""

import numpy as np

B, S, D, M = 4, 2048, 1024, 4
SQ = S // 2          # query rows per core
PART = 128
N_CORES = 8

_CACHE = {}


def build(nc_factory=None, S=S, D=D, SQ=SQ, M=M, use_deps=True):
    from concourse import bass, mybir, bacc, tile
    from concourse.tile import add_dep_helper

    fp32 = mybir.dt.float32
    bf16 = mybir.dt.bfloat16
    u8 = mybir.dt.uint8
    AF = mybir.ActivationFunctionType
    ALU = mybir.AluOpType

    P = PART
    DCH = D // P         # d-chunks (8)
    KCH = S // P         # key-row chunks (16)
    QTILES = SQ // P     # q-tiles per core (8)
    NB = 512
    GB = 2 * PART     # AV group width (2 q-tiles)

    if nc_factory is None:
        nc = bacc.Bacc("TRN2", target_bir_lowering=False, debug=False,
                       num_devices=N_CORES)
    else:
        nc = nc_factory()

    xT_d = nc.dram_tensor("xT", [D, S], bf16, kind="ExternalInput")
    mk_d = nc.dram_tensor("mk", [QTILES, P, M * S], u8, kind="ExternalInput")
    wq_d = nc.dram_tensor("Wq", [D, D], bf16, kind="ExternalInput")
    wk_d = nc.dram_tensor("Wk", [D, D], bf16, kind="ExternalInput")
    wv_d = nc.dram_tensor("Wv", [D, D], bf16, kind="ExternalInput")
    wo_d = nc.dram_tensor("Wo", [D, D], bf16, kind="ExternalInput")
    out_d = nc.dram_tensor("out", [SQ, D], bf16, kind="ExternalOutput")

    with tile.TileContext(nc) as tc:
        with tc.tile_pool(name="persist", bufs=1) as pp, \
             tc.tile_pool(name="psum", bufs=6, space="PSUM") as psp, \
             tc.tile_pool(name="psav", bufs=2, space="PSUM") as psav:

            QT = pp.tile([P, DCH * SQ], bf16, tag="QT")  # [p, j*SQ+q] = Q[q, j*128+p]
            KT = pp.tile([P, DCH * S], bf16, tag="KT")  # [p,j*S+k] = K[k,j*128+p]
            V = pp.tile([P, KCH * D], bf16)      # [p, i*D+d]  = V[i*128+p, d]

            def wload(dst, src_d):
                return nc.gpsimd.dma_start(
                    dst[:].rearrange("p (c d) -> p c d", c=DCH),
                    src_d.ap().rearrange("(c p) d -> p c d", p=P))

            # PSUM -> SBUF copy engines, round-robined so no engine stalls
            # the tensor stream (GPSIMD cannot access PSUM).
            cp_engines = [nc.vector, nc.scalar]
            cp_idx = [0]

            def psum_copy(dst_ap, src_ap):
                eng = cp_engines[cp_idx[0] % 2]
                cp_idx[0] += 1
                if eng is nc.scalar:
                    eng.copy(dst_ap, src_ap)
                else:
                    eng.tensor_copy(dst_ap, src_ap)

            def proj(dst, w_sb, src_sb, ncols, src_off=0):
                # dst[p, j*ncols+r] = sum_dx W[dx, j*128+p] * src[dx, src_off+r]
                for j in range(DCH):
                    for qb in range(ncols // NB):
                        ps = psp.tile([P, NB], fp32, tag="ps", name="ps")
                        for c in range(DCH):
                            nc.tensor.matmul(
                                ps[:],
                                w_sb[:, c * D + j * P: c * D + (j + 1) * P],
                                src_sb[:, c * S + src_off + qb * NB:
                                       c * S + src_off + (qb + 1) * NB],
                                start=(c == 0), stop=(c == DCH - 1))
                        psum_copy(
                            dst[:, j * ncols + qb * NB: j * ncols + (qb + 1) * NB],
                            ps[:])

            # ---- load + Q/K projections --------------------------------
            sx_ctx = tc.tile_pool(name="stage_x", bufs=1)
            sx = sx_ctx.__enter__()
            XT = sx.tile([P, DCH * S], bf16, name="XT")
            wqk_ctx = tc.tile_pool(name="stage_wqk", bufs=1)
            swqk = wqk_ctx.__enter__()
            Wq = swqk.tile([P, DCH * D], bf16, name="Wq")
            Wk = swqk.tile([P, DCH * D], bf16, name="Wk")
            # Lead-in: Wq rides the sync trigger queue in two j-halves,
            # concurrent with the XT chain on gpsimd (also split), so the
            # first Q-projection group starts after ~1.25MB of DMA.
            HD = (DCH // 2) * P
            d_wq0 = nc.sync.dma_start(
                Wq[:].rearrange("p (c d) -> p c d", c=DCH)[:, :, 0:HD],
                wq_d.ap().rearrange("(c p) d -> p c d", p=P)[:, :, 0:HD])
            d_wq1 = nc.sync.dma_start(
                Wq[:].rearrange("p (c d) -> p c d", c=DCH)[:, :, HD:D],
                wq_d.ap().rearrange("(c p) d -> p c d", p=P)[:, :, HD:D])
            d_xh = nc.gpsimd.dma_start(
                XT[:].rearrange("p (c r) -> p c r", c=DCH)[:, :, 0:NB],
                xT_d.ap().rearrange("(c p) r -> p c r", p=P)[:, :, 0:NB])
            d_xh2 = nc.gpsimd.dma_start(
                XT[:].rearrange("p (c r) -> p c r", c=DCH)[:, :, NB:SQ],
                xT_d.ap().rearrange("(c p) r -> p c r", p=P)[:, :, NB:SQ])
            d_xt = nc.gpsimd.dma_start(
                XT[:].rearrange("p (c r) -> p c r", c=DCH)[:, :, SQ:S],
                xT_d.ap().rearrange("(c p) r -> p c r", p=P)[:, :, SQ:S])
            d_wk = wload(Wk, wk_d)
            if use_deps:
                add_dep_helper(d_wq1.ins, d_wq0.ins, sync=False, reason="dma order")
                add_dep_helper(d_xh2.ins, d_xh.ins, sync=False, reason="dma order")
                add_dep_helper(d_xt.ins, d_xh2.ins, sync=False, reason="dma order")
                add_dep_helper(d_wk.ins, d_xt.ins, sync=False, reason="dma order")

            # warm the exp activation table during the projection phase
            with tc.tile_pool(name="warm", bufs=1) as wpool:
                wt = wpool.tile([P, 2], fp32)
                nc.vector.memset(wt[:], 0.0)
                nc.scalar.activation(wt[:], wt[:], AF.Exp)

            proj(QT, Wq, XT, SQ, src_off=0)
            proj(KT, Wk, XT, S, src_off=0)
            wqk_ctx.__exit__(None, None, None)

            # Wv reuses the Wq/Wk space (pool opened after wqk closes); its
            # load waits for the last Wq/Wk reader automatically.
            wv_ctx = tc.tile_pool(name="stage_wv", bufs=1)
            swv = wv_ctx.__enter__()
            Wv = swv.tile([P, DCH * D], bf16, name="Wv")
            d_wv = wload(Wv, wv_d)
            if use_deps:
                add_dep_helper(d_wv.ins, d_wk.ins, sync=False, reason="dma order")
            d_prev = d_wv

            # ---- work pools for the softmax/AV/out phases ----------------
            wk_ctx = tc.tile_pool(name="work", bufs=2)
            wkp = wk_ctx.__enter__()

            # ---- scores -> P -> masked softmax -> WsumT ------------------
            # V projection i-chunks are interleaved into the loop (2 per
            # q-tile) so the tensor engine fills the DVE-paced gaps.
            # Chains for the last two q-tiles are deferred into the AV
            # phase, where DVE/ACT are otherwise idle.
            inv_scale = 1.0 / float(np.sqrt(np.float32(D)))
            wtg_tiles = []
            tile_io = []

            den_tiles = {}

            def den_part(t):
                mt, Pt = tile_io[t]
                den = wkp.tile([P, M], fp32, tag="den", name="den")
                den_tiles[t] = den
                # fused product + row-sum per mask, in-place T_m = mask_m*P
                # (all on DVE: STT only exists there and GPSIMD TT is 2x
                # slower than the whole DVE chain).
                for m in range(M):
                    nc.vector.scalar_tensor_tensor(
                        out=mt[:, m * S:(m + 1) * S],
                        in0=mt[:, m * S:(m + 1) * S],
                        scalar=1.0, in1=Pt[:],
                        op0=ALU.mult, op1=ALU.mult,
                        accum_out=den[:, m:m + 1])
                # in-place reciprocal: den becomes inv
                nc.vector.reciprocal(den[:], den[:])

            def combine_part(t):
                mt, Pt = tile_io[t]
                inv = den_tiles[t]

                # Wsum = (inv0*T0 + inv1*T1) + (inv2*T2 + inv3*T3):
                #   ACT does the two scaled copies, DVE the two fused
                #   scaled adds (STT), GPSIMD the final add (off the
                #   critical path: only the transpose consumes it).
                A = wkp.tile([P, S], bf16, tag="A", name="A")
                C = wkp.tile([P, S], bf16, tag="C", name="C", bufs=1)
                nc.scalar.activation(A[:], mt[:, 0:S], AF.Copy,
                                     scale=inv[:, 0:1])
                nc.vector.scalar_tensor_tensor(
                    out=A[:], in0=mt[:, S:2 * S], scalar=inv[:, 1:2],
                    in1=A[:], op0=ALU.mult, op1=ALU.add)
                nc.scalar.activation(C[:], mt[:, 2 * S:3 * S], AF.Copy,
                                     scale=inv[:, 2:3])
                nc.vector.scalar_tensor_tensor(
                    out=C[:], in0=mt[:, 3 * S:4 * S], scalar=inv[:, 3:4],
                    in1=C[:], op0=ALU.mult, op1=ALU.add)
                nc.gpsimd.tensor_tensor(A[:], A[:], C[:], op=ALU.add)

                # transpose Wsum [128, S] -> WTg columns via xbar DMA
                wtg = wtg_tiles[t // 2]
                lt = (t % 2) * P
                nc.sync.dma_start_transpose(
                    wtg[:].rearrange("p (i q) -> p i q", i=KCH)[:, :, lt:lt + P],
                    A[:])
            for t in range(QTILES):
                mt = wkp.tile([P, M * S], bf16, tag="mt", name="mt")
                d_mt = nc.gpsimd.dma_start(mt[:], mk_d.ap()[t])
                if use_deps:
                    add_dep_helper(d_mt.ins, d_prev.ins, sync=False,
                                   reason="mask order")
                    d_prev = d_mt

                Pt = wkp.tile([P, S], bf16, tag="Pt", name="Pt")
                for kb in range(S // NB):
                    ps = psp.tile([P, NB], fp32, tag="ps", name="ps")
                    for c in range(DCH):
                        nc.tensor.matmul(
                            ps[:],
                            QT[:, c * SQ + t * P: c * SQ + (t + 1) * P],
                            KT[:, c * S + kb * NB: c * S + (kb + 1) * NB],
                            start=(c == 0), stop=(c == DCH - 1))
                    nc.scalar.activation(
                        Pt[:, kb * NB:(kb + 1) * NB], ps[:],
                        AF.Exp, scale=inv_scale)

                for i in (2 * t, 2 * t + 1):
                    for db in range(D // NB):
                        ps = psp.tile([P, NB], fp32, tag="ps", name="ps")
                        for c in range(DCH):
                            nc.tensor.matmul(
                                ps[:],
                                XT[:, c * S + i * P: c * S + (i + 1) * P],
                                Wv[:, c * D + db * NB: c * D + (db + 1) * NB],
                                start=(c == 0), stop=(c == DCH - 1))
                        # ScalarE only: a copy queued behind DVE STTs would
                        # hold the PSUM slot and stall the tensor stream.
                        nc.scalar.copy(
                            V[:, i * D + db * NB: i * D + (db + 1) * NB],
                            ps[:])

                if t % 2 == 0:
                    wtg_tiles.append(
                        wkp.tile([P, KCH * GB], bf16, tag="WTg", name="WTg",
                                 bufs=3))
                tile_io.append((mt, Pt))
                # software pipeline: den(t) first, then combine(t-1) —
                # den's DVE work covers the ACT scaled-copy latency of the
                # previous tile's combine, keeping DVE dense.
                den_part(t)
                if t >= 1:
                    combine_part(t - 1)

            # OT reuses QT's slot (QT dead after the last scores matmul);
            # same shape, so the tag alias is exact.
            OT = pp.tile([P, DCH * SQ], bf16, name="OT", tag="QT")
            #    [p, j*SQ+q] = out_pre[q, j*128+p]

            # Wo reuses KT's slot (KT dead after the last scores matmul);
            # issued after the mask DMAs so they are not delayed.
            Wo = pp.tile([P, DCH * D], bf16, name="Wo", tag="KT")
            d_wo = wload(Wo, wo_d)
            if use_deps:
                add_dep_helper(d_wo.ins, d_prev.ins, sync=False, reason="dma order")

            # ---- AV (2-tile groups) interleaved with the out projection --
            def g_tile(t):
                ot = wkp.tile([P, D], bf16, tag="ot", name="ot", bufs=1)
                for db in range(D // NB):
                    ps = psp.tile([P, NB], fp32, tag="ps", name="ps")
                    for c in range(DCH):
                        nc.tensor.matmul(
                            ps[:],
                            OT[:, c * SQ + t * P: c * SQ + (t + 1) * P],
                            Wo[:, c * D + db * NB: c * D + (db + 1) * NB],
                            start=(c == 0), stop=(c == DCH - 1))
                    psum_copy(ot[:, db * NB:(db + 1) * NB], ps[:])
                nc.sync.dma_start(out_d.ap()[t * P:(t + 1) * P, :], ot[:])

            for g in range(SQ // GB):
                if g == 0:
                    combine_part(QTILES - 1)   # deferred under AV g0
                for j in range(DCH):
                    ps = psav.tile([P, GB], fp32, tag="av", name="av")
                    for i in range(KCH):
                        nc.tensor.matmul(
                            ps[:],
                            V[:, i * D + j * P: i * D + (j + 1) * P],
                            wtg_tiles[g][:, i * GB:(i + 1) * GB],
                            start=(i == 0), stop=(i == KCH - 1))
                    psum_copy(
                        OT[:, j * SQ + g * GB: j * SQ + (g + 1) * GB],
                        ps[:])
                for t in range(g * GB // P, (g + 1) * GB // P):
                    g_tile(t)
            wk_ctx.__exit__(None, None, None)
            wv_ctx.__exit__(None, None, None)
            sx_ctx.__exit__(None, None, None)

    nc.compile()
    return nc


def _get_nc():
    if "nc" not in _CACHE:
        _CACHE["nc"] = build()
    return _CACHE["nc"]


def _prep_inputs(x, stride_masks, Wq, Wk, Wv, Wo):
    """Host-side dtype/layout prep (no math beyond the Wo * 1/M fold)."""
    from ml_dtypes import bfloat16

    QTILES = SQ // PART

    wq = np.ascontiguousarray(Wq.astype(bfloat16))
    wk = np.ascontiguousarray(Wk.astype(bfloat16))
    wv = np.ascontiguousarray(Wv.astype(bfloat16))
    wo = np.ascontiguousarray((Wo / np.float32(M)).astype(bfloat16))

    # xT per (batch, half): own query-half columns first (key permutation)
    xT = {}
    for b in range(B):
        xb = np.ascontiguousarray(x[b].T.astype(bfloat16))  # [D, S]
        xT[(b, 0)] = xb
        xT[(b, 1)] = np.ascontiguousarray(
            np.concatenate([xb[:, SQ:], xb[:, :SQ]], axis=1))

    # masks: uint8, per-half q slice, same key permutation, tile layout
    m8 = stride_masks.astype(np.uint8)  # [M, S, S]
    mk = {}
    for h in range(2):
        v = m8[:, h * SQ:(h + 1) * SQ, :]                    # [M, SQ, S]
        if h == 1:
            v = np.concatenate([v[:, :, SQ:], v[:, :, :SQ]], axis=2)
        v = v.transpose(1, 0, 2).reshape(QTILES, PART, M * S)
        mk[h] = np.ascontiguousarray(v)
    return wq, wk, wv, wo, xT, mk


def kernel(x, stride_masks, Wq, bq, Wk, bk, Wv, bv, Wo, bo):
    from concourse import bass_utils

    x = np.ascontiguousarray(np.asarray(x, dtype=np.float32))
    stride_masks = np.ascontiguousarray(np.asarray(stride_masks, dtype=np.int32))
    Wq = np.asarray(Wq, dtype=np.float32)
    Wk = np.asarray(Wk, dtype=np.float32)
    Wv = np.asarray(Wv, dtype=np.float32)
    Wo = np.asarray(Wo, dtype=np.float32)
    bq = np.asarray(bq, dtype=np.float32)
    bk = np.asarray(bk, dtype=np.float32)
    bv = np.asarray(bv, dtype=np.float32)
    bo = np.asarray(bo, dtype=np.float32)

    nc = _get_nc()

    # Biases are spec'd zero-fill; the device kernel omits them. bv/bo fold
    # in exactly on the host (softmax rows sum to 1); bq/bk would need a
    # device path, so assert they are zero.
    assert not (np.any(bq) or np.any(bk)), "nonzero q/k bias unsupported"

    wq, wk, wv, wo, xT, mk = _prep_inputs(x, stride_masks, Wq, Wk, Wv, Wo)

    in_maps = []
    for c in range(N_CORES):
        b, h = c // 2, c % 2
        in_maps.append({
            "xT": xT[(b, h)], "mk": mk[h],
            "Wq": wq, "Wk": wk, "Wv": wv, "Wo": wo,
        })

    res = bass_utils.run_bass_kernel_spmd(nc, in_maps, core_ids=list(range(N_CORES)))
    _CACHE["last_results"] = res

    out = np.empty((B, S, D), dtype=np.float32)
    for c in range(N_CORES):
        b, h = c // 2, c % 2
        out[b, h * SQ:(h + 1) * SQ, :] = np.asarray(
            res.results[c]["out"]).astype(np.float32)

    if np.any(bv):
        out += (bv @ Wo)[None, None, :]
    if np.any(bo):
        out += bo[None, None, :]
    return out


# revision 21
# speedup vs baseline: 1.0445x; 1.0445x over previous
"""CAMSA multi-mask attention kernel for one TRN2 chip (8 NeuronCores).

Problem: B=4, S=2048, D=1024, M=4 stride masks.
  Q = x@Wq + bq ; K = x@Wk + bk ; V = x@Wv + bv     (biases are zero-fill)
  scores = Q K^T / sqrt(D)                           [B,S,S]
  weights_m = softmax(where(mask_m==0, -1e9, scores))
  out = (mean_m weights_m) @ V @ Wo + bo

Algebra: with P = exp(scores/sqrt(D)) (no row-max needed; scores ~ N(0,1)):
  den_m[q] = sum_k mask_m[q,k] P[q,k];  inv_m = 1/den_m
  Wsum = sum_m inv_m * (mask_m*P);  out = Wsum @ V @ (Wo/M)
(the 1/M mean is folded into Wo on the host).

Sharding: core c = (batch b=c//2, query-half h=c%2): 1024 query rows,
full 2048 keys; K/V projections duplicated within a batch pair (no
collectives).  Host-side prep is pure dtype/layout: bf16 casts of
x/W (the device would DMA-cast anyway), masks int32 -> uint8 in a
per-q-tile layout, and a per-core "own half first" permutation of the
key axis (applied consistently to x columns and mask k) so one SPMD
program serves both halves without a separate xTq input.

Device pipeline per core (all matmuls bf16, contraction on partitions):
  tensor order: Q proj, K proj, V proj, scores t=0..7, AV 2-tile
  groups interleaved with the Wo projection per q-tile.
  per q-tile softmax chain under the matmul stream:
    ACT: P=exp(scores) from PSUM (4 blocks), 2 scaled copies
    DVE: 3x STT(mask*P, accum->den), recip, 2x TS(4x), 3x TT(2x)
    GpSimd: 1x STT(mask*P, accum->den)
    sync-DMA: Wsum -> WT transpose (xbar), outputs
"""

import numpy as np

B, S, D, M = 4, 2048, 1024, 4
SQ = S // 2          # query rows per core
PART = 128
N_CORES = 8

_CACHE = {}


def build(nc_factory=None, S=S, D=D, SQ=SQ, M=M, use_deps=True):
    from concourse import bass, mybir, bacc, tile
    from concourse.tile import add_dep_helper

    fp32 = mybir.dt.float32
    bf16 = mybir.dt.bfloat16
    u8 = mybir.dt.uint8
    AF = mybir.ActivationFunctionType
    ALU = mybir.AluOpType

    P = PART
    DCH = D // P         # d-chunks (8)
    KCH = S // P         # key-row chunks (16)
    QTILES = SQ // P     # q-tiles per core (8)
    NB = 512
    GB = 2 * PART     # AV group width (2 q-tiles)

    if nc_factory is None:
        nc = bacc.Bacc("TRN2", target_bir_lowering=False, debug=False,
                       num_devices=N_CORES)
    else:
        nc = nc_factory()

    xT_d = nc.dram_tensor("xT", [D, S], bf16, kind="ExternalInput")
    mk_d = nc.dram_tensor("mk", [QTILES, P, M * S], u8, kind="ExternalInput")
    wq_d = nc.dram_tensor("Wq", [D, D], bf16, kind="ExternalInput")
    wk_d = nc.dram_tensor("Wk", [D, D], bf16, kind="ExternalInput")
    wv_d = nc.dram_tensor("Wv", [D, D], bf16, kind="ExternalInput")
    wo_d = nc.dram_tensor("Wo", [D, D], bf16, kind="ExternalInput")
    out_d = nc.dram_tensor("out", [SQ, D], bf16, kind="ExternalOutput")

    with tile.TileContext(nc) as tc:
        with tc.tile_pool(name="persist", bufs=1) as pp, \
             tc.tile_pool(name="psum", bufs=6, space="PSUM") as psp, \
             tc.tile_pool(name="psav", bufs=2, space="PSUM") as psav:

            QT = pp.tile([P, DCH * SQ], bf16, tag="QT")  # [p, j*SQ+q] = Q[q, j*128+p]
            KT = pp.tile([P, DCH * S], bf16, tag="KT")  # [p,j*S+k] = K[k,j*128+p]
            V = pp.tile([P, KCH * D], bf16)      # [p, i*D+d]  = V[i*128+p, d]

            def wload(dst, src_d):
                return nc.gpsimd.dma_start(
                    dst[:].rearrange("p (c d) -> p c d", c=DCH),
                    src_d.ap().rearrange("(c p) d -> p c d", p=P))

            # PSUM -> SBUF copy engines, round-robined so no engine stalls
            # the tensor stream (GPSIMD cannot access PSUM).
            cp_engines = [nc.vector, nc.scalar]
            cp_idx = [0]

            def psum_copy(dst_ap, src_ap):
                eng = cp_engines[cp_idx[0] % 2]
                cp_idx[0] += 1
                if eng is nc.scalar:
                    eng.copy(dst_ap, src_ap)
                else:
                    eng.tensor_copy(dst_ap, src_ap)

            def proj(dst, w_sb, src_sb, ncols, src_off=0):
                # dst[p, j*ncols+r] = sum_dx W[dx, j*128+p] * src[dx, src_off+r]
                for j in range(DCH):
                    for qb in range(ncols // NB):
                        ps = psp.tile([P, NB], fp32, tag="ps", name="ps")
                        for c in range(DCH):
                            nc.tensor.matmul(
                                ps[:],
                                w_sb[:, c * D + j * P: c * D + (j + 1) * P],
                                src_sb[:, c * S + src_off + qb * NB:
                                       c * S + src_off + (qb + 1) * NB],
                                start=(c == 0), stop=(c == DCH - 1))
                        psum_copy(
                            dst[:, j * ncols + qb * NB: j * ncols + (qb + 1) * NB],
                            ps[:])

            # ---- load + Q/K projections --------------------------------
            sx_ctx = tc.tile_pool(name="stage_x", bufs=1)
            sx = sx_ctx.__enter__()
            XT = sx.tile([P, DCH * S], bf16, name="XT")
            wqk_ctx = tc.tile_pool(name="stage_wqk", bufs=1)
            swqk = wqk_ctx.__enter__()
            Wq = swqk.tile([P, DCH * D], bf16, name="Wq")
            Wk = swqk.tile([P, DCH * D], bf16, name="Wk")
            # Lead-in: Wq rides the sync trigger queue in two j-halves,
            # concurrent with the XT chain on gpsimd (also split), so the
            # first Q-projection group starts after ~1.25MB of DMA.
            HD = (DCH // 2) * P
            d_wq0 = nc.sync.dma_start(
                Wq[:].rearrange("p (c d) -> p c d", c=DCH)[:, :, 0:HD],
                wq_d.ap().rearrange("(c p) d -> p c d", p=P)[:, :, 0:HD])
            d_wq1 = nc.sync.dma_start(
                Wq[:].rearrange("p (c d) -> p c d", c=DCH)[:, :, HD:D],
                wq_d.ap().rearrange("(c p) d -> p c d", p=P)[:, :, HD:D])
            d_xh = nc.gpsimd.dma_start(
                XT[:].rearrange("p (c r) -> p c r", c=DCH)[:, :, 0:NB],
                xT_d.ap().rearrange("(c p) r -> p c r", p=P)[:, :, 0:NB])
            d_xh2 = nc.gpsimd.dma_start(
                XT[:].rearrange("p (c r) -> p c r", c=DCH)[:, :, NB:SQ],
                xT_d.ap().rearrange("(c p) r -> p c r", p=P)[:, :, NB:SQ])
            d_xt = nc.gpsimd.dma_start(
                XT[:].rearrange("p (c r) -> p c r", c=DCH)[:, :, SQ:S],
                xT_d.ap().rearrange("(c p) r -> p c r", p=P)[:, :, SQ:S])
            d_wk = wload(Wk, wk_d)
            if use_deps:
                add_dep_helper(d_wq1.ins, d_wq0.ins, sync=False, reason="dma order")
                add_dep_helper(d_xh2.ins, d_xh.ins, sync=False, reason="dma order")
                add_dep_helper(d_xt.ins, d_xh2.ins, sync=False, reason="dma order")
                add_dep_helper(d_wk.ins, d_xt.ins, sync=False, reason="dma order")

            # warm the exp activation table during the projection phase
            with tc.tile_pool(name="warm", bufs=1) as wpool:
                wt = wpool.tile([P, 2], fp32)
                nc.vector.memset(wt[:], 0.0)
                nc.scalar.activation(wt[:], wt[:], AF.Exp)

            proj(QT, Wq, XT, SQ, src_off=0)
            proj(KT, Wk, XT, S, src_off=0)
            wqk_ctx.__exit__(None, None, None)

            # Wv reuses the Wq/Wk space (pool opened after wqk closes); its
            # load waits for the last Wq/Wk reader automatically.
            wv_ctx = tc.tile_pool(name="stage_wv", bufs=1)
            swv = wv_ctx.__enter__()
            Wv = swv.tile([P, DCH * D], bf16, name="Wv")
            d_wv = wload(Wv, wv_d)
            if use_deps:
                add_dep_helper(d_wv.ins, d_wk.ins, sync=False, reason="dma order")
            d_prev = d_wv

            # ---- work pools for the softmax/AV/out phases ----------------
            wk_ctx = tc.tile_pool(name="work", bufs=2)
            wkp = wk_ctx.__enter__()

            # ---- scores -> P -> masked softmax -> WsumT ------------------
            # V projection i-chunks are interleaved into the loop (2 per
            # q-tile) so the tensor engine fills the DVE-paced gaps.
            # Chains for the last two q-tiles are deferred into the AV
            # phase, where DVE/ACT are otherwise idle.
            inv_scale = 1.0 / float(np.sqrt(np.float32(D)))
            wtg_tiles = []
            tile_io = []

            def chain(t):
                mtm, Pt = tile_io[t]
                den = wkp.tile([P, M], fp32, tag="den", name="den")
                # fused product + row-sum per mask, in-place T_m = mask_m*P
                # (all on DVE: STT only exists there and GPSIMD TT is 2x
                # slower than the whole DVE chain).
                for m in range(M):
                    nc.vector.scalar_tensor_tensor(
                        out=mtm[m][:],
                        in0=mtm[m][:],
                        scalar=1.0, in1=Pt[:],
                        op0=ALU.mult, op1=ALU.mult,
                        accum_out=den[:, m:m + 1])
                # in-place reciprocal: den becomes inv
                nc.vector.reciprocal(den[:], den[:])
                inv = den

                # Wsum = (inv0*T0 + inv1*T1) + (inv2*T2 + inv3*T3):
                #   ACT does the two scaled copies, DVE the two fused
                #   scaled adds (STT), GPSIMD the final add (off the
                #   critical path: only the transpose consumes it).
                A = wkp.tile([P, S], bf16, tag="A", name="A")
                C = wkp.tile([P, S], bf16, tag="C", name="C", bufs=1)
                nc.scalar.activation(A[:], mtm[0][:], AF.Copy,
                                     scale=inv[:, 0:1])
                nc.vector.scalar_tensor_tensor(
                    out=A[:], in0=mtm[1][:], scalar=inv[:, 1:2],
                    in1=A[:], op0=ALU.mult, op1=ALU.add)
                nc.scalar.activation(C[:], mtm[2][:], AF.Copy,
                                     scale=inv[:, 2:3])
                nc.vector.scalar_tensor_tensor(
                    out=C[:], in0=mtm[3][:], scalar=inv[:, 3:4],
                    in1=C[:], op0=ALU.mult, op1=ALU.add)
                nc.gpsimd.tensor_tensor(A[:], A[:], C[:], op=ALU.add)

                # transpose Wsum [128, S] -> WTg columns via xbar DMA
                wtg = wtg_tiles[t // 2]
                lt = (t % 2) * P
                nc.sync.dma_start_transpose(
                    wtg[:].rearrange("p (i q) -> p i q", i=KCH)[:, :, lt:lt + P],
                    A[:])
            for t in range(QTILES):
                mtm = [wkp.tile([P, S], bf16, tag=f"mt{m}", name=f"mt{m}")
                       for m in range(M)]
                for m in range(M):
                    d_mt = nc.gpsimd.dma_start(
                        mtm[m][:], mk_d.ap()[t][:, m * S:(m + 1) * S])
                    if use_deps:
                        add_dep_helper(d_mt.ins, d_prev.ins, sync=False,
                                       reason="mask order")
                        d_prev = d_mt

                Pt = wkp.tile([P, S], bf16, tag="Pt", name="Pt")
                for kb in range(S // NB):
                    ps = psp.tile([P, NB], fp32, tag="ps", name="ps")
                    for c in range(DCH):
                        nc.tensor.matmul(
                            ps[:],
                            QT[:, c * SQ + t * P: c * SQ + (t + 1) * P],
                            KT[:, c * S + kb * NB: c * S + (kb + 1) * NB],
                            start=(c == 0), stop=(c == DCH - 1))
                    nc.scalar.activation(
                        Pt[:, kb * NB:(kb + 1) * NB], ps[:],
                        AF.Exp, scale=inv_scale)

                for i in (2 * t, 2 * t + 1):
                    for db in range(D // NB):
                        ps = psp.tile([P, NB], fp32, tag="ps", name="ps")
                        for c in range(DCH):
                            nc.tensor.matmul(
                                ps[:],
                                XT[:, c * S + i * P: c * S + (i + 1) * P],
                                Wv[:, c * D + db * NB: c * D + (db + 1) * NB],
                                start=(c == 0), stop=(c == DCH - 1))
                        # ScalarE only: a copy queued behind DVE STTs would
                        # hold the PSUM slot and stall the tensor stream.
                        nc.scalar.copy(
                            V[:, i * D + db * NB: i * D + (db + 1) * NB],
                            ps[:])

                if t % 2 == 0:
                    wtg_tiles.append(
                        wkp.tile([P, KCH * GB], bf16, tag="WTg", name="WTg",
                                 bufs=3))
                tile_io.append((mtm, Pt))
                if t < QTILES - 2:
                    chain(t)

            # OT reuses QT's slot (QT dead after the last scores matmul);
            # same shape, so the tag alias is exact.
            OT = pp.tile([P, DCH * SQ], bf16, name="OT", tag="QT")
            #    [p, j*SQ+q] = out_pre[q, j*128+p]

            # Wo reuses KT's slot (KT dead after the last scores matmul);
            # issued after the mask DMAs so they are not delayed.
            Wo = pp.tile([P, DCH * D], bf16, name="Wo", tag="KT")
            d_wo = wload(Wo, wo_d)
            if use_deps:
                add_dep_helper(d_wo.ins, d_prev.ins, sync=False, reason="dma order")

            # ---- AV (2-tile groups) interleaved with the out projection --
            def g_tile(t):
                ot = wkp.tile([P, D], bf16, tag="ot", name="ot", bufs=1)
                for db in range(D // NB):
                    ps = psp.tile([P, NB], fp32, tag="ps", name="ps")
                    for c in range(DCH):
                        nc.tensor.matmul(
                            ps[:],
                            OT[:, c * SQ + t * P: c * SQ + (t + 1) * P],
                            Wo[:, c * D + db * NB: c * D + (db + 1) * NB],
                            start=(c == 0), stop=(c == DCH - 1))
                    psum_copy(ot[:, db * NB:(db + 1) * NB], ps[:])
                nc.sync.dma_start(out_d.ap()[t * P:(t + 1) * P, :], ot[:])

            for g in range(SQ // GB):
                if g < 2:
                    chain(QTILES - 2 + g)   # deferred chains under AV g0/g1
                for j in range(DCH):
                    ps = psav.tile([P, GB], fp32, tag="av", name="av")
                    for i in range(KCH):
                        nc.tensor.matmul(
                            ps[:],
                            V[:, i * D + j * P: i * D + (j + 1) * P],
                            wtg_tiles[g][:, i * GB:(i + 1) * GB],
                            start=(i == 0), stop=(i == KCH - 1))
                    psum_copy(
                        OT[:, j * SQ + g * GB: j * SQ + (g + 1) * GB],
                        ps[:])
                for t in range(g * GB // P, (g + 1) * GB // P):
                    g_tile(t)
            wk_ctx.__exit__(None, None, None)
            wv_ctx.__exit__(None, None, None)
            sx_ctx.__exit__(None, None, None)

    nc.compile()
    return nc


def _get_nc():
    if "nc" not in _CACHE:
        _CACHE["nc"] = build()
    return _CACHE["nc"]


def _prep_inputs(x, stride_masks, Wq, Wk, Wv, Wo):
    """Host-side dtype/layout prep (no math beyond the Wo * 1/M fold)."""
    from ml_dtypes import bfloat16

    QTILES = SQ // PART

    wq = np.ascontiguousarray(Wq.astype(bfloat16))
    wk = np.ascontiguousarray(Wk.astype(bfloat16))
    wv = np.ascontiguousarray(Wv.astype(bfloat16))
    wo = np.ascontiguousarray((Wo / np.float32(M)).astype(bfloat16))

    # xT per (batch, half): own query-half columns first (key permutation)
    xT = {}
    for b in range(B):
        xb = np.ascontiguousarray(x[b].T.astype(bfloat16))  # [D, S]
        xT[(b, 0)] = xb
        xT[(b, 1)] = np.ascontiguousarray(
            np.concatenate([xb[:, SQ:], xb[:, :SQ]], axis=1))

    # masks: uint8, per-half q slice, same key permutation, tile layout
    m8 = stride_masks.astype(np.uint8)  # [M, S, S]
    mk = {}
    for h in range(2):
        v = m8[:, h * SQ:(h + 1) * SQ, :]                    # [M, SQ, S]
        if h == 1:
            v = np.concatenate([v[:, :, SQ:], v[:, :, :SQ]], axis=2)
        v = v.transpose(1, 0, 2).reshape(QTILES, PART, M * S)
        mk[h] = np.ascontiguousarray(v)
    return wq, wk, wv, wo, xT, mk


def kernel(x, stride_masks, Wq, bq, Wk, bk, Wv, bv, Wo, bo):
    from concourse import bass_utils

    x = np.ascontiguousarray(np.asarray(x, dtype=np.float32))
    stride_masks = np.ascontiguousarray(np.asarray(stride_masks, dtype=np.int32))
    Wq = np.asarray(Wq, dtype=np.float32)
    Wk = np.asarray(Wk, dtype=np.float32)
    Wv = np.asarray(Wv, dtype=np.float32)
    Wo = np.asarray(Wo, dtype=np.float32)
    bq = np.asarray(bq, dtype=np.float32)
    bk = np.asarray(bk, dtype=np.float32)
    bv = np.asarray(bv, dtype=np.float32)
    bo = np.asarray(bo, dtype=np.float32)

    nc = _get_nc()

    # Biases are spec'd zero-fill; the device kernel omits them. bv/bo fold
    # in exactly on the host (softmax rows sum to 1); bq/bk would need a
    # device path, so assert they are zero.
    assert not (np.any(bq) or np.any(bk)), "nonzero q/k bias unsupported"

    wq, wk, wv, wo, xT, mk = _prep_inputs(x, stride_masks, Wq, Wk, Wv, Wo)

    in_maps = []
    for c in range(N_CORES):
        b, h = c // 2, c % 2
        in_maps.append({
            "xT": xT[(b, h)], "mk": mk[h],
            "Wq": wq, "Wk": wk, "Wv": wv, "Wo": wo,
        })

    res = bass_utils.run_bass_kernel_spmd(nc, in_maps, core_ids=list(range(N_CORES)))
    _CACHE["last_results"] = res

    out = np.empty((B, S, D), dtype=np.float32)
    for c in range(N_CORES):
        b, h = c // 2, c % 2
        out[b, h * SQ:(h + 1) * SQ, :] = np.asarray(
            res.results[c]["out"]).astype(np.float32)

    if np.any(bv):
        out += (bv @ Wo)[None, None, :]
    if np.any(bo):
        out += bo[None, None, :]
    return out


# revision 22
# speedup vs baseline: 1.0579x; 1.0129x over previous
"""CAMSA multi-mask attention kernel for one TRN2 chip (8 NeuronCores).

Problem: B=4, S=2048, D=1024, M=4 stride masks.
  Q = x@Wq + bq ; K = x@Wk + bk ; V = x@Wv + bv     (biases are zero-fill)
  scores = Q K^T / sqrt(D)                           [B,S,S]
  weights_m = softmax(where(mask_m==0, -1e9, scores))
  out = (mean_m weights_m) @ V @ Wo + bo

Algebra: with P = exp(scores/sqrt(D)) (no row-max needed; scores ~ N(0,1)):
  den_m[q] = sum_k mask_m[q,k] P[q,k];  inv_m = 1/den_m
  Wsum = sum_m inv_m * (mask_m*P);  out = Wsum @ V @ (Wo/M)
(the 1/M mean is folded into Wo on the host).

Sharding: core c = (batch b=c//2, query-half h=c%2): 1024 query rows,
full 2048 keys; K/V projections duplicated within a batch pair (no
collectives).  Host-side prep is pure dtype/layout: bf16 casts of
x/W (the device would DMA-cast anyway), masks int32 -> uint8 in a
per-q-tile layout, and a per-core "own half first" permutation of the
key axis (applied consistently to x columns and mask k) so one SPMD
program serves both halves without a separate xTq input.

Device pipeline per core (all matmuls bf16, contraction on partitions):
  tensor order: Q proj, K proj, V proj, scores t=0..7, AV 2-tile
  groups interleaved with the Wo projection per q-tile.
  per q-tile softmax chain under the matmul stream:
    ACT: P=exp(scores) from PSUM (4 blocks), 2 scaled copies
    DVE: 3x STT(mask*P, accum->den), recip, 2x TS(4x), 3x TT(2x)
    GpSimd: 1x STT(mask*P, accum->den)
    sync-DMA: Wsum -> WT transpose (xbar), outputs
"""

import numpy as np

B, S, D, M = 4, 2048, 1024, 4
SQ = S // 2          # query rows per core
PART = 128
N_CORES = 8

_CACHE = {}


def build(nc_factory=None, S=S, D=D, SQ=SQ, M=M, use_deps=True):
    from concourse import bass, mybir, bacc, tile
    from concourse.tile import add_dep_helper

    fp32 = mybir.dt.float32
    bf16 = mybir.dt.bfloat16
    u8 = mybir.dt.uint8
    AF = mybir.ActivationFunctionType
    ALU = mybir.AluOpType

    P = PART
    DCH = D // P         # d-chunks (8)
    KCH = S // P         # key-row chunks (16)
    QTILES = SQ // P     # q-tiles per core (8)
    NB = 512
    GB = 2 * PART     # AV group width (2 q-tiles)

    if nc_factory is None:
        nc = bacc.Bacc("TRN2", target_bir_lowering=False, debug=False,
                       num_devices=N_CORES)
    else:
        nc = nc_factory()

    xT_d = nc.dram_tensor("xT", [D, S], bf16, kind="ExternalInput")
    mk_d = nc.dram_tensor("mk", [QTILES, P, M * S], u8, kind="ExternalInput")
    wq_d = nc.dram_tensor("Wq", [D, D], bf16, kind="ExternalInput")
    wk_d = nc.dram_tensor("Wk", [D, D], bf16, kind="ExternalInput")
    wv_d = nc.dram_tensor("Wv", [D, D], bf16, kind="ExternalInput")
    wo_d = nc.dram_tensor("Wo", [D, D], bf16, kind="ExternalInput")
    out_d = nc.dram_tensor("out", [SQ, D], bf16, kind="ExternalOutput")

    with tile.TileContext(nc) as tc:
        with tc.tile_pool(name="persist", bufs=1) as pp, \
             tc.tile_pool(name="psum", bufs=6, space="PSUM") as psp, \
             tc.tile_pool(name="psav", bufs=2, space="PSUM") as psav:

            QT = pp.tile([P, DCH * SQ], bf16, tag="QT")  # [p, j*SQ+q] = Q[q, j*128+p]
            KT = pp.tile([P, DCH * S], bf16, tag="KT")  # [p,j*S+k] = K[k,j*128+p]
            V = pp.tile([P, KCH * D], bf16)      # [p, i*D+d]  = V[i*128+p, d]

            def wload(dst, src_d):
                return nc.gpsimd.dma_start(
                    dst[:].rearrange("p (c d) -> p c d", c=DCH),
                    src_d.ap().rearrange("(c p) d -> p c d", p=P))

            # PSUM -> SBUF copy engines, round-robined so no engine stalls
            # the tensor stream (GPSIMD cannot access PSUM).
            cp_engines = [nc.vector, nc.scalar]
            cp_idx = [0]

            def psum_copy(dst_ap, src_ap):
                eng = cp_engines[cp_idx[0] % 2]
                cp_idx[0] += 1
                if eng is nc.scalar:
                    eng.copy(dst_ap, src_ap)
                else:
                    eng.tensor_copy(dst_ap, src_ap)

            def proj(dst, w_sb, src_sb, ncols, src_off=0):
                # dst[p, j*ncols+r] = sum_dx W[dx, j*128+p] * src[dx, src_off+r]
                for j in range(DCH):
                    for qb in range(ncols // NB):
                        ps = psp.tile([P, NB], fp32, tag="ps", name="ps")
                        for c in range(DCH):
                            nc.tensor.matmul(
                                ps[:],
                                w_sb[:, c * D + j * P: c * D + (j + 1) * P],
                                src_sb[:, c * S + src_off + qb * NB:
                                       c * S + src_off + (qb + 1) * NB],
                                start=(c == 0), stop=(c == DCH - 1))
                        psum_copy(
                            dst[:, j * ncols + qb * NB: j * ncols + (qb + 1) * NB],
                            ps[:])

            # ---- load + Q/K projections --------------------------------
            sx_ctx = tc.tile_pool(name="stage_x", bufs=1)
            sx = sx_ctx.__enter__()
            XT = sx.tile([P, DCH * S], bf16, name="XT")
            wqk_ctx = tc.tile_pool(name="stage_wqk", bufs=1)
            swqk = wqk_ctx.__enter__()
            Wq = swqk.tile([P, DCH * D], bf16, name="Wq")
            Wk = swqk.tile([P, DCH * D], bf16, name="Wk")
            # Lead-in: Wq rides the sync trigger queue in two j-halves,
            # concurrent with the XT chain on gpsimd (also split), so the
            # first Q-projection group starts after ~1.25MB of DMA.
            HD = (DCH // 2) * P
            d_wq0 = nc.sync.dma_start(
                Wq[:].rearrange("p (c d) -> p c d", c=DCH)[:, :, 0:HD],
                wq_d.ap().rearrange("(c p) d -> p c d", p=P)[:, :, 0:HD])
            d_wq1 = nc.sync.dma_start(
                Wq[:].rearrange("p (c d) -> p c d", c=DCH)[:, :, HD:D],
                wq_d.ap().rearrange("(c p) d -> p c d", p=P)[:, :, HD:D])
            d_xh = nc.gpsimd.dma_start(
                XT[:].rearrange("p (c r) -> p c r", c=DCH)[:, :, 0:NB],
                xT_d.ap().rearrange("(c p) r -> p c r", p=P)[:, :, 0:NB])
            d_xh2 = nc.gpsimd.dma_start(
                XT[:].rearrange("p (c r) -> p c r", c=DCH)[:, :, NB:SQ],
                xT_d.ap().rearrange("(c p) r -> p c r", p=P)[:, :, NB:SQ])
            d_xt = nc.gpsimd.dma_start(
                XT[:].rearrange("p (c r) -> p c r", c=DCH)[:, :, SQ:S],
                xT_d.ap().rearrange("(c p) r -> p c r", p=P)[:, :, SQ:S])
            d_wk = wload(Wk, wk_d)
            if use_deps:
                add_dep_helper(d_wq1.ins, d_wq0.ins, sync=False, reason="dma order")
                add_dep_helper(d_xh2.ins, d_xh.ins, sync=False, reason="dma order")
                add_dep_helper(d_xt.ins, d_xh2.ins, sync=False, reason="dma order")
                add_dep_helper(d_wk.ins, d_xt.ins, sync=False, reason="dma order")

            # warm the exp activation table during the projection phase
            with tc.tile_pool(name="warm", bufs=1) as wpool:
                wt = wpool.tile([P, 2], fp32)
                nc.vector.memset(wt[:], 0.0)
                nc.scalar.activation(wt[:], wt[:], AF.Exp)

            proj(QT, Wq, XT, SQ, src_off=0)
            proj(KT, Wk, XT, S, src_off=0)
            wqk_ctx.__exit__(None, None, None)

            # Wv reuses the Wq/Wk space (pool opened after wqk closes); its
            # load waits for the last Wq/Wk reader automatically.
            wv_ctx = tc.tile_pool(name="stage_wv", bufs=1)
            swv = wv_ctx.__enter__()
            Wv = swv.tile([P, DCH * D], bf16, name="Wv")
            d_wv = wload(Wv, wv_d)
            if use_deps:
                add_dep_helper(d_wv.ins, d_wk.ins, sync=False, reason="dma order")
            d_prev = d_wv

            # ---- work pools for the softmax/AV/out phases ----------------
            wk_ctx = tc.tile_pool(name="work", bufs=2)
            wkp = wk_ctx.__enter__()

            # ---- scores -> P -> masked softmax -> WsumT ------------------
            # V projection i-chunks are interleaved into the loop (2 per
            # q-tile) so the tensor engine fills the DVE-paced gaps.
            # Chains for the last two q-tiles are deferred into the AV
            # phase, where DVE/ACT are otherwise idle.
            inv_scale = 1.0 / float(np.sqrt(np.float32(D)))
            wtg_tiles = []
            tile_io = []

            def chain(t):
                mtm, Pt = tile_io[t]
                den = wkp.tile([P, M], fp32, tag="den", name="den")
                # fused product + row-sum per mask, in-place T_m = mask_m*P
                # (all on DVE: STT only exists there and GPSIMD TT is 2x
                # slower than the whole DVE chain).
                for m in range(M):
                    nc.vector.scalar_tensor_tensor(
                        out=mtm[m][:],
                        in0=mtm[m][:],
                        scalar=1.0, in1=Pt[:],
                        op0=ALU.mult, op1=ALU.mult,
                        accum_out=den[:, m:m + 1])
                # in-place reciprocal: den becomes inv
                nc.vector.reciprocal(den[:], den[:])
                inv = den

                # Wsum = (inv0*T0 + inv1*T1) + (inv2*T2 + inv3*T3):
                #   ACT does the two scaled copies, DVE the two fused
                #   scaled adds (STT), GPSIMD the final add (off the
                #   critical path: only the transpose consumes it).
                A = wkp.tile([P, S], bf16, tag="A", name="A")
                C = wkp.tile([P, S], bf16, tag="C", name="C", bufs=1)
                nc.vector.tensor_scalar(A[:], mtm[0][:],
                                        inv[:, 0:1], None, ALU.mult)
                nc.vector.scalar_tensor_tensor(
                    out=A[:], in0=mtm[1][:], scalar=inv[:, 1:2],
                    in1=A[:], op0=ALU.mult, op1=ALU.add)
                nc.vector.tensor_scalar(C[:], mtm[2][:],
                                        inv[:, 2:3], None, ALU.mult)
                nc.vector.scalar_tensor_tensor(
                    out=C[:], in0=mtm[3][:], scalar=inv[:, 3:4],
                    in1=C[:], op0=ALU.mult, op1=ALU.add)
                nc.gpsimd.tensor_tensor(A[:], A[:], C[:], op=ALU.add)

                # transpose Wsum [128, S] -> WTg columns via xbar DMA
                wtg = wtg_tiles[t // 2]
                lt = (t % 2) * P
                nc.sync.dma_start_transpose(
                    wtg[:].rearrange("p (i q) -> p i q", i=KCH)[:, :, lt:lt + P],
                    A[:])
            for t in range(QTILES):
                mtm = [wkp.tile([P, S], bf16, tag=f"mt{m}", name=f"mt{m}")
                       for m in range(M)]
                for m in range(M):
                    d_mt = nc.gpsimd.dma_start(
                        mtm[m][:], mk_d.ap()[t][:, m * S:(m + 1) * S])
                    if use_deps:
                        add_dep_helper(d_mt.ins, d_prev.ins, sync=False,
                                       reason="mask order")
                        d_prev = d_mt

                Pt = wkp.tile([P, S], bf16, tag="Pt", name="Pt")
                for kb in range(S // NB):
                    ps = psp.tile([P, NB], fp32, tag="ps", name="ps")
                    for c in range(DCH):
                        nc.tensor.matmul(
                            ps[:],
                            QT[:, c * SQ + t * P: c * SQ + (t + 1) * P],
                            KT[:, c * S + kb * NB: c * S + (kb + 1) * NB],
                            start=(c == 0), stop=(c == DCH - 1))
                    nc.scalar.activation(
                        Pt[:, kb * NB:(kb + 1) * NB], ps[:],
                        AF.Exp, scale=inv_scale)

                for i in (2 * t, 2 * t + 1):
                    for db in range(D // NB):
                        ps = psp.tile([P, NB], fp32, tag="ps", name="ps")
                        for c in range(DCH):
                            nc.tensor.matmul(
                                ps[:],
                                XT[:, c * S + i * P: c * S + (i + 1) * P],
                                Wv[:, c * D + db * NB: c * D + (db + 1) * NB],
                                start=(c == 0), stop=(c == DCH - 1))
                        # ScalarE only: a copy queued behind DVE STTs would
                        # hold the PSUM slot and stall the tensor stream.
                        nc.scalar.copy(
                            V[:, i * D + db * NB: i * D + (db + 1) * NB],
                            ps[:])

                if t % 2 == 0:
                    wtg_tiles.append(
                        wkp.tile([P, KCH * GB], bf16, tag="WTg", name="WTg",
                                 bufs=3))
                tile_io.append((mtm, Pt))
                if t < QTILES - 2:
                    chain(t)

            # OT reuses QT's slot (QT dead after the last scores matmul);
            # same shape, so the tag alias is exact.
            OT = pp.tile([P, DCH * SQ], bf16, name="OT", tag="QT")
            #    [p, j*SQ+q] = out_pre[q, j*128+p]

            # Wo reuses KT's slot (KT dead after the last scores matmul);
            # issued after the mask DMAs so they are not delayed.
            Wo = pp.tile([P, DCH * D], bf16, name="Wo", tag="KT")
            d_wo = wload(Wo, wo_d)
            if use_deps:
                add_dep_helper(d_wo.ins, d_prev.ins, sync=False, reason="dma order")

            # ---- AV (2-tile groups) interleaved with the out projection --
            def g_tile(t):
                ot = wkp.tile([P, D], bf16, tag="ot", name="ot", bufs=1)
                for db in range(D // NB):
                    ps = psp.tile([P, NB], fp32, tag="ps", name="ps")
                    for c in range(DCH):
                        nc.tensor.matmul(
                            ps[:],
                            OT[:, c * SQ + t * P: c * SQ + (t + 1) * P],
                            Wo[:, c * D + db * NB: c * D + (db + 1) * NB],
                            start=(c == 0), stop=(c == DCH - 1))
                    psum_copy(ot[:, db * NB:(db + 1) * NB], ps[:])
                nc.sync.dma_start(out_d.ap()[t * P:(t + 1) * P, :], ot[:])

            for g in range(SQ // GB):
                if g < 2:
                    chain(QTILES - 2 + g)   # deferred chains under AV g0/g1
                for j in range(DCH):
                    ps = psav.tile([P, GB], fp32, tag="av", name="av")
                    for i in range(KCH):
                        nc.tensor.matmul(
                            ps[:],
                            V[:, i * D + j * P: i * D + (j + 1) * P],
                            wtg_tiles[g][:, i * GB:(i + 1) * GB],
                            start=(i == 0), stop=(i == KCH - 1))
                    psum_copy(
                        OT[:, j * SQ + g * GB: j * SQ + (g + 1) * GB],
                        ps[:])
                for t in range(g * GB // P, (g + 1) * GB // P):
                    g_tile(t)
            wk_ctx.__exit__(None, None, None)
            wv_ctx.__exit__(None, None, None)
            sx_ctx.__exit__(None, None, None)

    nc.compile()
    return nc


def _get_nc():
    if "nc" not in _CACHE:
        _CACHE["nc"] = build()
    return _CACHE["nc"]


def _prep_inputs(x, stride_masks, Wq, Wk, Wv, Wo):
    """Host-side dtype/layout prep (no math beyond the Wo * 1/M fold)."""
    from ml_dtypes import bfloat16

    QTILES = SQ // PART

    wq = np.ascontiguousarray(Wq.astype(bfloat16))
    wk = np.ascontiguousarray(Wk.astype(bfloat16))
    wv = np.ascontiguousarray(Wv.astype(bfloat16))
    wo = np.ascontiguousarray((Wo / np.float32(M)).astype(bfloat16))

    # xT per (batch, half): own query-half columns first (key permutation)
    xT = {}
    for b in range(B):
        xb = np.ascontiguousarray(x[b].T.astype(bfloat16))  # [D, S]
        xT[(b, 0)] = xb
        xT[(b, 1)] = np.ascontiguousarray(
            np.concatenate([xb[:, SQ:], xb[:, :SQ]], axis=1))

    # masks: uint8, per-half q slice, same key permutation, tile layout
    m8 = stride_masks.astype(np.uint8)  # [M, S, S]
    mk = {}
    for h in range(2):
        v = m8[:, h * SQ:(h + 1) * SQ, :]                    # [M, SQ, S]
        if h == 1:
            v = np.concatenate([v[:, :, SQ:], v[:, :, :SQ]], axis=2)
        v = v.transpose(1, 0, 2).reshape(QTILES, PART, M * S)
        mk[h] = np.ascontiguousarray(v)
    return wq, wk, wv, wo, xT, mk


def kernel(x, stride_masks, Wq, bq, Wk, bk, Wv, bv, Wo, bo):
    from concourse import bass_utils

    x = np.ascontiguousarray(np.asarray(x, dtype=np.float32))
    stride_masks = np.ascontiguousarray(np.asarray(stride_masks, dtype=np.int32))
    Wq = np.asarray(Wq, dtype=np.float32)
    Wk = np.asarray(Wk, dtype=np.float32)
    Wv = np.asarray(Wv, dtype=np.float32)
    Wo = np.asarray(Wo, dtype=np.float32)
    bq = np.asarray(bq, dtype=np.float32)
    bk = np.asarray(bk, dtype=np.float32)
    bv = np.asarray(bv, dtype=np.float32)
    bo = np.asarray(bo, dtype=np.float32)

    nc = _get_nc()

    # Biases are spec'd zero-fill; the device kernel omits them. bv/bo fold
    # in exactly on the host (softmax rows sum to 1); bq/bk would need a
    # device path, so assert they are zero.
    assert not (np.any(bq) or np.any(bk)), "nonzero q/k bias unsupported"

    wq, wk, wv, wo, xT, mk = _prep_inputs(x, stride_masks, Wq, Wk, Wv, Wo)

    in_maps = []
    for c in range(N_CORES):
        b, h = c // 2, c % 2
        in_maps.append({
            "xT": xT[(b, h)], "mk": mk[h],
            "Wq": wq, "Wk": wk, "Wv": wv, "Wo": wo,
        })

    res = bass_utils.run_bass_kernel_spmd(nc, in_maps, core_ids=list(range(N_CORES)))
    _CACHE["last_results"] = res

    out = np.empty((B, S, D), dtype=np.float32)
    for c in range(N_CORES):
        b, h = c // 2, c % 2
        out[b, h * SQ:(h + 1) * SQ, :] = np.asarray(
            res.results[c]["out"]).astype(np.float32)

    if np.any(bv):
        out += (bv @ Wo)[None, None, :]
    if np.any(bo):
        out += bo[None, None, :]
    return out
